# revision 1
# baseline (speedup 1.0000x reference)
"""AWD-LSTM forward on 8 Trainium2 NeuronCores.

Sharding: 8 cores = 4 pairs; even core of a pair runs the forward
direction, odd the backward (fed time-reversed inputs so both are
forward scans).  Batch (32) is sharded 8 rows per pair.  The layer-1
input needs both directions' layer-0 output sequences, exchanged
between partners with a pairwise AllGather (shipped pre-reversed so the
receiver can read it in its own scan order).  The tiny fc+mish head
runs on host.

Device notes (see recurrence_b for the active scan implementation):
  - all matmul operands are bf16 (fp32 runs the PE in a 2-pass LOW_HI
    mode at ~1/3 throughput); accumulation stays fp32 in PSUM.
  - the recurrence is weight-stationary with all state transposed
    [128, 4*8]: per 4H-chunk, stationary = W_hh^T tile [128, 128] bf16,
    moving = h^T chunk [128, 8]; the pre-activation row is folded into
    PSUM by a transpose-style matmul (pre chunk stationary, identity
    moving).  i/f and g/o gates use separate PSUM banks so their
    activations overlap the remaining matmuls; h^T is produced directly
    in next-step matmul layout (no transposes anywhere).
  - hidden sequences are stored transposed-by-chunk [128, 4, T*8] so
    the per-step store is one DMA per target and layer-1's stationary
    loads are contiguous.  Stores read a rotating staging tile (never
    the live h^T — its async DMA read racing the next step's overwrite
    was a real hardware corruption) and split across the Sync and
    GpSimd queues.
  - measured: 16.28 ms HW exec, 2.9e-3 relative absmax vs fp32
    reference.  Bound by PE instruction issue: 80 LDWEIGHTS+MATMUL
    pairs/step at ~77 ns over 2048 serial steps.  Known-bad variants:
    staggered_reset (+3 ms), unroll 16 (+2 ms), fp32 operands (~3x),
    batch-rows gate layout (HAM throttling + 863 ns activations).
"""

import os
import sys
import numpy as np

sys.path.insert(0, "/opt/trn_rl_repo")

import concourse.bass as bass
import concourse.bacc as bacc
import concourse.mybir as mybir
import concourse.tile as tile
from concourse.bass import ds
from concourse.masks import make_identity

B, S, E = 32, 1024, 512
H = E
G4 = 4 * H            # 2048
BS = 8                # batch rows per pair
NCORES = 8
F32 = mybir.dt.float32
BF16 = mybir.dt.bfloat16

AF = mybir.ActivationFunctionType


def build_program(T=S, unroll=8):
    """Build the SPMD Bass program (identical on all cores)."""
    TB = T * BS
    nc = bacc.Bacc(None, target_bir_lowering=False)

    # ---- I/O ----
    x_T = nc.dram_tensor("x_T", [4, 128, TB], BF16, kind="ExternalInput")
    w0T = nc.dram_tensor("w0T", [4, 128, G4], BF16, kind="ExternalInput")
    bias0 = nc.dram_tensor("bias0", [1, G4], BF16, kind="ExternalInput")
    whh0T = nc.dram_tensor("whh0T", [4, 128, G4], BF16, kind="ExternalInput")
    w1oT = nc.dram_tensor("w1oT", [4, 128, G4], BF16, kind="ExternalInput")
    w1pT = nc.dram_tensor("w1pT", [4, 128, G4], BF16, kind="ExternalInput")
    bias1 = nc.dram_tensor("bias1", [1, G4], BF16, kind="ExternalInput")
    whh1T = nc.dram_tensor("whh1T", [4, 128, G4], BF16, kind="ExternalInput")
    pslot = nc.dram_tensor("pslot", [1, 1], mybir.dt.uint32, kind="ExternalInput")
    h1 = nc.dram_tensor("h1", [128, 32], BF16, kind="ExternalOutput")
    dbg_pre0 = nc.dram_tensor("dbg_pre0", [128, G4], BF16, kind="ExternalOutput")
    dbg_loc = nc.dram_tensor("dbg_loc", [128, 4, 64], BF16, kind="ExternalOutput")
    dbg_ag = nc.dram_tensor("dbg_ag", [2, 128, 4, 64], BF16, kind="ExternalOutput")
    dbg_pre1 = nc.dram_tensor("dbg_pre1", [128, G4], BF16, kind="ExternalOutput")

    with tile.TileContext(nc) as tc:
        # ---- internal DRAM (pool tiles so deps are tracked) ----
        with tc.tile_pool(name="dram", bufs=1, space="DRAM") as dram:
            pre0 = dram.tile([TB, G4], BF16)
            pre1 = dram.tile([TB, G4], BF16)
            out0_locT = dram.tile([128, 4, TB], BF16)
            out0_revT = dram.tile([128, 4, TB], BF16)
            ag_out = dram.tile([2, 128, 4, TB], BF16)

            # ---- constants ----
            with tc.tile_pool(name="const", bufs=1) as cpool:
                ones1 = cpool.tile([1, 128], BF16)
                nc.gpsimd.memset(ones1[:], 1.0)
                i8b = cpool.tile([8, 8], BF16)
                make_identity(nc, i8b[:])
                i8f = cpool.tile([8, 8], F32)
                make_identity(nc, i8f[:])

                # =========== P1: pre0 = x @ W_ih0^T + bias0 ===========
                projection(tc, pre0, [(x_T, w0T)], bias0, ones1, TB)

                # =========== P2: layer-0 recurrence ===========
                recurrence_b(tc, pre0, whh0T, i8b, T, unroll,
                             out_locT=out0_locT, out_revT=out0_revT)

                # =========== exchange: pairwise AllGather of reversed seq ====
                nc.gpsimd.collective_compute(
                    "AllGather",
                    mybir.AluOpType.bypass,
                    ins=[out0_revT.opt()],
                    outs=[ag_out.opt()],
                    replica_groups=[[0, 1], [2, 3], [4, 5], [6, 7]],
                )

                # =========== P3: pre1 = own@W1o^T + partner@W1p^T + bias1 ====
                with tc.tile_pool(name="pslot", bufs=1) as pp:
                    pslot_sb = pp.tile([1, 1], mybir.dt.uint32)
                    nc.sync.dma_start(pslot_sb[:], pslot[:])
                    tmp_reg = nc.sync.alloc_register("pslot_reg")
                    nc.sync.reg_load(tmp_reg, pslot_sb[0:1, 0:1])
                    slot_reg = nc.sync.snap(tmp_reg, donate=True, min_val=0, max_val=1)

                    projection(tc, pre1,
                               [(out0_locT, w1oT), ((ag_out, slot_reg), w1pT)],
                               bias1, ones1, TB)

                # =========== P4: layer-1 recurrence (no sequence output) =====
                h_last = recurrence_b(tc, pre1, whh1T, i8b, T, unroll)

                nc.sync.dma_start(h1[:], h_last[:])
                # debug taps (tiny DRAM->DRAM copies)
                nc.sync.dma_start(dbg_pre0[:], pre0[0:128, :])
                nc.sync.dma_start(dbg_loc[:], out0_locT[:, :, 0:64])
                nc.sync.dma_start(dbg_ag[:], ag_out[:, :, :, 0:64])
                nc.sync.dma_start(dbg_pre1[:], pre1[0:128, :])

    nc.compile()
    return nc


def projection(tc, dst, sources, bias_dram, ones1, TB):
    """dst[TB, G4](bf16) = bias + sum_i src_i @ W_i  (operands pre-transposed).

    sources: list of (srcT, wT); srcT is [4,128,TB] (x_T input) or
    [128,4,TB] (hidden-seq layout) or ((tensor [2,128,4,TB], slot_reg))
    for the AllGather output indexed dynamically.
    """
    nc = tc.nc
    n_src = len(sources)
    with (
        tc.tile_pool(name="pw", bufs=1) as wpool,
        tc.tile_pool(name="px", bufs=3) as xpool,
        tc.tile_pool(name="pout", bufs=3) as opool,
        tc.tile_pool(name="pb", bufs=1) as bpool,
        tc.tile_pool(name="pps", bufs=2, space="PSUM") as pspool,
    ):
        bias_sb = bpool.tile([1, G4], BF16)
        nc.sync.dma_start(bias_sb[:], bias_dram[:])
        w_sb = []
        for si, (_, wT) in enumerate(sources):
            for k in range(4):
                w = wpool.tile([128, G4], BF16, name=f"w{si}_{k}")
                nc.sync.dma_start(w[:], wT[k])
                w_sb.append(w)

        for m in range(TB // 128):
            c0 = m * 128
            x_sb = []
            for si, (srcT, _) in enumerate(sources):
                for k in range(4):
                    xt = xpool.tile([128, 128], BF16, name=f"x{si}_{k}", tag=f"x{si}_{k}")
                    if isinstance(srcT, tuple):
                        t_, reg = srcT
                        nc.sync.dma_start(
                            xt[:],
                            t_[ds(reg, 1), :, k, c0:c0 + 128].rearrange("s p c -> (s p) c"),
                        )
                    elif len(srcT.shape) == 3 and srcT.shape[0] == 4:
                        nc.sync.dma_start(xt[:], srcT[k, :, c0:c0 + 128])
                    else:
                        nc.sync.dma_start(xt[:], srcT[:, k, c0:c0 + 128])
                    x_sb.append(xt)

            ps = pspool.tile([128, G4], F32, tag="ps")
            for n in range(4):
                ns = slice(n * 512, (n + 1) * 512)
                nc.tensor.matmul(ps[:, ns], ones1[:, :128], bias_sb[0:1, ns],
                                 start=True, stop=False)
                for i in range(4 * n_src):
                    nc.tensor.matmul(ps[:, ns], x_sb[i][:], w_sb[i][:, ns],
                                     start=False, stop=(i == 4 * n_src - 1))
            o_sb = opool.tile([128, G4], BF16, tag="o")
            nc.vector.tensor_copy(o_sb[:], ps[:])
            nc.sync.dma_start(dst[c0:c0 + 128, :], o_sb[:])


def recurrence_b(tc, pre, whhT_dram, i8b, T, unroll, out_locT=None, out_revT=None):
    """Weight-stationary LSTM scan; all state transposed [128, 4*8].

    Gates are computed as g^T tiles: for 4H-chunk m, PSUM [128, 8] slices
    accumulate sum_k W^T[k-chunk, m-chunk].T @ h^T[k-chunk] plus a
    pre-activation fold (pre chunk as stationary, identity moving).
    i/f gates land in one PSUM bank, g/o in another, so the i/f
    activations overlap the g/o matmuls.  h^T is produced directly in
    the layout the next step's matmuls (and the sequence stores) need —
    no transposes anywhere.  Returns hT_all [128, 32] bf16.
    """
    nc = tc.nc
    store = out_locT is not None
    with (
        tc.tile_pool(name="rw", bufs=1) as wpool,
        tc.tile_pool(name="rstate", bufs=1) as spool,
        tc.tile_pool(name="rpre", bufs=2 * unroll) as prepool,
        tc.tile_pool(name="rgate", bufs=2) as gpool,
        tc.tile_pool(name="rst", bufs=4) as stpool,
        tc.tile_pool(name="rps", bufs=2, space="PSUM") as gps_pool,
    ):
        whh_sb = []
        for k in range(4):
            w = wpool.tile([128, G4], BF16, name=f"whh{k}")
            nc.sync.dma_start(w[:], whhT_dram[k])
            whh_sb.append(w)

        cT = spool.tile([128, 32], F32)
        hT_all = spool.tile([128, 32], BF16)
        nc.gpsimd.memset(cT[:], 0.0)
        nc.gpsimd.memset(hT_all[:], 0.0)

        REV = 8 * T - 8
        with tc.For_i(0, 8 * T, 8 * unroll,
                      hint_engines=(mybir.EngineType.PE,)) as iv0:
            for u in range(unroll):
                iv8 = iv0 + 8 * u
                pre_sb = prepool.tile([8, G4], BF16, tag="pre")
                nc.sync.dma_start(pre_sb[:], pre[ds(iv8, 8), :])

                ps_if = gps_pool.tile([128, 64], F32, name="psif", tag="psif")
                ps_go = gps_pool.tile([128, 64], F32, name="psgo", tag="psgo")
                # m-chunk order: i(0-3), f(4-7) into ps_if; g(8-11), o(12-15)
                for m in range(16):
                    ps = ps_if if m < 8 else ps_go
                    col = slice(8 * (m % 8), 8 * (m % 8) + 8)
                    ms = slice(128 * m, 128 * (m + 1))
                    nc.tensor.matmul(ps[:, col], pre_sb[:, ms], i8b[:],
                                     start=True, stop=False)
                    for k in range(4):
                        nc.tensor.matmul(ps[:, col], whh_sb[k][:, ms],
                                         hT_all[:, 8 * k:8 * k + 8],
                                         start=False, stop=(k == 3))

                si = gpool.tile([128, 32], F32, tag="si")
                nc.scalar.activation(si[:], ps_if[:, 0:32], AF.Sigmoid)
                sf = gpool.tile([128, 32], F32, tag="sf")
                nc.scalar.activation(sf[:], ps_if[:, 32:64], AF.Sigmoid)
                tg = gpool.tile([128, 32], F32, tag="tg")
                nc.scalar.activation(tg[:], ps_go[:, 0:32], AF.Tanh)
                so = gpool.tile([128, 32], F32, tag="so")
                nc.scalar.activation(so[:], ps_go[:, 32:64], AF.Sigmoid)

                a = gpool.tile([128, 32], F32, tag="a")
                nc.vector.tensor_tensor(a[:], si[:], tg[:], mybir.AluOpType.mult)
                btmp = gpool.tile([128, 32], F32, tag="b")
                nc.vector.tensor_tensor(btmp[:], sf[:], cT[:], mybir.AluOpType.mult)
                nc.vector.tensor_tensor(cT[:], a[:], btmp[:], mybir.AluOpType.add)
                tcT = gpool.tile([128, 32], F32, tag="tc")
                nc.scalar.activation(tcT[:], cT[:], AF.Tanh)
                nc.vector.tensor_tensor(hT_all[:], so[:], tcT[:],
                                        mybir.AluOpType.mult)

                if store:
                    st = stpool.tile([128, 32], BF16, tag="st")
                    nc.vector.tensor_copy(st[:], hT_all[:])
                    st3 = st[:].rearrange("p (k b) -> p k b", k=4)
                    nc.sync.dma_start(out_locT[:, :, ds(iv8, 8)], st3)
                    nc.gpsimd.dma_start(out_revT[:, :, ds(REV - iv8, 8)], st3)
        return hT_all


def recurrence(tc, pre, whhT_dram, i8b, i8f, T, unroll, out_locT=None, out_revT=None):
    """LSTM scan over pre[T*8, G4] (bf16).  Returns persistent h tile [8, 512] f32."""
    nc = tc.nc
    store = out_locT is not None
    with (
        tc.tile_pool(name="rw", bufs=1) as wpool,
        tc.tile_pool(name="rstate", bufs=1) as spool,
        tc.tile_pool(name="rpre", bufs=2 * unroll) as prepool,
        tc.tile_pool(name="rgate", bufs=2) as gpool,
        tc.tile_pool(name="rst", bufs=4) as stpool,
        tc.tile_pool(name="rps", bufs=1, space="PSUM") as gps_pool,
        tc.tile_pool(name="rtp", bufs=2, space="PSUM") as tp_pool,
    ):
        whh_sb = []
        for k in range(4):
            w = wpool.tile([128, G4], BF16, name=f"whh{k}")
            nc.sync.dma_start(w[:], whhT_dram[k])
            whh_sb.append(w)

        c_st = spool.tile([8, H], F32)
        h_st = spool.tile([8, H], F32)
        hT_all = spool.tile([128, 32], BF16)
        nc.gpsimd.memset(c_st[:], 0.0)
        nc.gpsimd.memset(hT_all[:], 0.0)

        REV = 8 * T - 8
        with tc.For_i(0, 8 * T, 8 * unroll) as iv0:
            for u in range(unroll):
                iv8 = iv0 + 8 * u
                pre_sb = prepool.tile([8, G4], BF16, tag="pre")
                nc.sync.dma_start(pre_sb[:], pre[ds(iv8, 8), :])

                # gate banks: i, f, g, o — each its own PSUM bank
                gates = []
                for n in range(4):
                    ns = slice(n * 512, (n + 1) * 512)
                    g_ps = gps_pool.tile([8, 512], F32, name=f"gp{n}", tag=f"gp{n}")
                    nc.tensor.matmul(g_ps[:], i8b[:], pre_sb[:, ns],
                                     start=True, stop=False)
                    for k in range(4):
                        nc.tensor.matmul(g_ps[:], hT_all[:, 8 * k:8 * k + 8],
                                         whh_sb[k][:, ns], start=False, stop=(k == 3))
                    act = gpool.tile([8, 512], F32, name=f"ga{n}", tag=f"ga{n}")
                    fn = AF.Tanh if n == 2 else AF.Sigmoid
                    nc.scalar.activation(act[:], g_ps[:], fn)
                    gates.append(act)

                si, sf, tg, so = gates
                a = gpool.tile([8, 512], F32, tag="a")
                nc.vector.tensor_tensor(a[:], si[:], tg[:], mybir.AluOpType.mult)
                btmp = gpool.tile([8, 512], F32, tag="b")
                nc.vector.tensor_tensor(btmp[:], sf[:], c_st[:], mybir.AluOpType.mult)
                nc.vector.tensor_tensor(c_st[:], a[:], btmp[:], mybir.AluOpType.add)
                tc_t = gpool.tile([8, 512], F32, tag="tc")
                nc.scalar.activation(tc_t[:], c_st[:], AF.Tanh)
                nc.vector.tensor_tensor(h_st[:], so[:], tc_t[:], mybir.AluOpType.mult)

                tp_ps = tp_pool.tile([128, 32], F32, tag="tp")
                for k in range(4):
                    nc.tensor.transpose(tp_ps[:, 8 * k:8 * k + 8],
                                        h_st[:, 128 * k:128 * (k + 1)], i8f[:])
                nc.vector.tensor_copy(hT_all[:], tp_ps[:])  # f32 -> bf16 cast

                if store:
                    # staged copy: the DMA reads a rotating slot that is not
                    # rewritten for another 4 steps (no WAR race with hT_all)
                    st = stpool.tile([128, 32], BF16, tag="st")
                    nc.vector.tensor_copy(st[:], tp_ps[:])
                    st3 = st[:].rearrange("p (k b) -> p k b", k=4)
                    nc.gpsimd.dma_start(out_locT[:, :, ds(iv8, 8)], st3)
                    nc.gpsimd.dma_start(out_revT[:, :, ds(REV - iv8, 8)], st3)
        return h_st


# ----------------------------------------------------------------------------
# Host side
# ----------------------------------------------------------------------------

_PROG_CACHE = {}


def _get_program(T):
    if T not in _PROG_CACHE:
        _PROG_CACHE[T] = build_program(T)
    return _PROG_CACHE[T]


def _bf16(a):
    import ml_dtypes
    return np.asarray(a, np.float32).astype(ml_dtypes.bfloat16)


def _prep_inputs(x, w_ih_f0, w_hh_f0, b_ih_f0, b_hh_f0,
                 w_ih_b0, w_hh_b0, b_ih_b0, b_hh_b0,
                 w_ih_f1, w_hh_f1, b_ih_f1, b_hh_f1,
                 w_ih_b1, w_hh_b1, b_ih_b1, b_hh_b1,
                 mask, T):
    """Build the 8 per-core input maps."""
    f32 = np.float32

    def chunkT(w):  # [G4, K] weight -> W^T as bf16 [K//128, 128, G4]
        wt = np.ascontiguousarray(w.T.astype(f32))          # [K, G4]
        return _bf16(wt.reshape(wt.shape[0] // 128, 128, w.shape[0]))

    whh_f0m = (w_hh_f0 * mask).astype(f32)

    per_dir = {
        0: dict(w0T=chunkT(w_ih_f0), bias0=_bf16(b_ih_f0 + b_hh_f0)[None],
                whh0T=chunkT(whh_f0m),
                w1oT=chunkT(w_ih_f1[:, :H]), w1pT=chunkT(w_ih_f1[:, H:]),
                bias1=_bf16(b_ih_f1 + b_hh_f1)[None],
                whh1T=chunkT(w_hh_f1)),
        1: dict(w0T=chunkT(w_ih_b0), bias0=_bf16(b_ih_b0 + b_hh_b0)[None],
                whh0T=chunkT(w_hh_b0),
                w1oT=chunkT(w_ih_b1[:, H:]), w1pT=chunkT(w_ih_b1[:, :H]),
                bias1=_bf16(b_ih_b1 + b_hh_b1)[None],
                whh1T=chunkT(w_hh_b1)),
    }

    in_maps = []
    for core in range(NCORES):
        pair, q = core // 2, core % 2
        xs = x[pair * BS:(pair + 1) * BS, :T].astype(f32)   # [8, T, E]
        if q == 1:
            xs = xs[:, ::-1]
        # -> [E, T, 8] -> [4, 128, T*8]
        xT = _bf16(np.ascontiguousarray(xs.transpose(2, 1, 0)).reshape(4, 128, T * BS))
        m = dict(per_dir[q])
        m["x_T"] = xT
        m["pslot"] = np.array([[1 - q]], dtype=np.uint32)
        in_maps.append(m)
    return in_maps


def _mish(x):
    return x * np.tanh(np.log1p(np.exp(-np.abs(x))) + np.maximum(x, 0.0))


def _unT(hT):
    """[128, 32] h^T-chunk layout -> [8, 512]: h[b, 128k+p] = hT[p, 8k+b]."""
    a = np.asarray(hT, np.float32).reshape(128, 4, 8)     # p, k, b
    return np.ascontiguousarray(a.transpose(2, 1, 0)).reshape(8, 512)


def _head(h1s, fc_w, fc_b):
    """h1s: list of 8 per-core h^T arrays (core order). Returns [32, 512]."""
    h1s = [_unT(h) for h in h1s]
    h_f = np.concatenate([np.asarray(h1s[2 * p], np.float32) for p in range(4)], axis=0)
    h_b = np.concatenate([np.asarray(h1s[2 * p + 1], np.float32) for p in range(4)], axis=0)
    h = 0.5 * (h_f + h_b)
    z = h @ np.asarray(fc_w, np.float32).T + np.asarray(fc_b, np.float32)
    return _mish(z).astype(np.float32)


def run_device(inputs, trace=False, tmpdir=None):
    """Run the device portion; returns (h1s, BassKernelResults)."""
    from concourse.bass_utils import run_bass_kernel_spmd

    x = inputs["x"]
    T = x.shape[1]
    nc = _get_program(T)
    in_maps = _prep_inputs(
        x, inputs["w_ih_f0"], inputs["w_hh_f0"], inputs["b_ih_f0"], inputs["b_hh_f0"],
        inputs["w_ih_b0"], inputs["w_hh_b0"], inputs["b_ih_b0"], inputs["b_hh_b0"],
        inputs["w_ih_f1"], inputs["w_hh_f1"], inputs["b_ih_f1"], inputs["b_hh_f1"],
        inputs["w_ih_b1"], inputs["w_hh_b1"], inputs["b_ih_b1"], inputs["b_hh_b1"],
        inputs["mask"], T)

    res = run_bass_kernel_spmd(nc, in_maps, list(range(NCORES)),
                               trace=trace, tmpdir=tmpdir)
    h1s = [res.results[c]["h1"] for c in range(NCORES)]
    return h1s, res


def kernel(**inputs):
    h1s, _ = run_device(inputs)
    return _head(h1s, inputs["fc_w"], inputs["fc_b"])



# revision 10
# speedup vs baseline: 1.7135x; 1.7135x over previous
"""AWD-LSTM forward on 8 Trainium2 NeuronCores — v2 "warm" design.

Sharding (unchanged from v1): 8 cores = 4 pairs; even core of a pair runs
the forward direction, odd the backward (fed time-reversed inputs so both
are forward scans).  Batch (32) is sharded 8 rows per pair.  Layer-1 input
needs both directions' layer-0 sequences, exchanged with a pairwise
AllGather.  The tiny fc+mish head runs on host.

v2 changes (vs the 16.3 ms v1):
  - The input projections (pre = x @ W_ih^T + bias) are FUSED into the
    recurrence loop: each step's tail (the ACT/DVE gate chain, ~1.5 us of
    PE idle in v1) is filled with a few projection matmuls that compute
    pre for steps ~16-32 ahead, into SBUF ping-pong chunk buffers.  This
    keeps the PE busy so the HAM clock gate stays at K=8/8 (2.4 GHz) —
    v1 ran throttled at 1.2 GHz for 94% of the kernel — and absorbs the
    ~0.7 ms standalone projection phases.
  - The 16 per-step pre-fold transpose-matmuls are gone: pre enters the
    gate PSUM banks via a DVE value-overwrite while the accumulation
    group stays open forever (one dummy start=True matmul at init; all
    gate matmuls start=False).  has_written bits stay set, so matmuls
    accumulate on top of the DVE-written pre values.
  - W_hh is fp8-e4m3 (stationary side only; h stays bf16): FWL loads
    fp8 weights 2x faster than bf16.  Everything is pre-scaled by 16 on
    host (clears e4m3's subnormal floor) and descaled for free via the
    ACT affine (scale=1/16).  Measured fp8 impact on final error vs
    fp32 reference: 6.7e-3 (budget 2e-2).
  - Gate banks laid out [i|g] and [f|o] so the exposed tail chain is
    sigmoid(f,o) -> f*c -> +i*tanh(g) -> tanh(c) -> o*tanh(c), with the
    i/g activations hidden under the f/o matmuls.
"""

import sys

sys.path.insert(0, "/opt/trn_rl_repo")

import numpy as np

import concourse.bass as bass
import concourse.bacc as bacc
import concourse.mybir as mybir
import concourse.tile as tile
from concourse.bass import ds

B, S, E = 32, 1024, 512
H = E
G4 = 4 * H            # 2048
BS = 8                # batch rows per pair
NCORES = 8
CH = 16               # steps per pre chunk
UNROLL = 32           # steps per loop body (2 chunks)
PAD = 2 * CH * BS     # x/seq column padding (2 chunks of lookahead slop)
F32 = mybir.dt.float32
BF16 = mybir.dt.bfloat16
FP8 = mybir.dt.float8e4
USE_FP8 = True

AF = mybir.ActivationFunctionType
MULT = mybir.AluOpType.mult
ADD = mybir.AluOpType.add

# mslot s (production/bias/pre-buffer order) -> real m-chunk (gate dim / 128)
# slots 0-7 -> bank_ig [i0..3, g0..3]; slots 8-15 -> bank_fo [f0..3, o0..3]
MORDER = [0, 1, 2, 3, 8, 9, 10, 11, 4, 5, 6, 7, 12, 13, 14, 15]


def build_program(T=S, fp8=USE_FP8):
    TB = T * BS
    TBP = TB + PAD
    WDT = FP8 if fp8 else BF16
    nc = bacc.Bacc(None, target_bir_lowering=False)

    # ---- I/O ----
    x_T = nc.dram_tensor("x_T", [4, 128, TBP], BF16, kind="ExternalInput")
    w0T = nc.dram_tensor("w0T", [4, 128, G4], BF16, kind="ExternalInput")
    b0T = nc.dram_tensor("b0T", [128, 16], F32, kind="ExternalInput")
    whh0T = nc.dram_tensor("whh0T", [4, 128, G4], WDT, kind="ExternalInput")
    w1oT = nc.dram_tensor("w1oT", [4, 128, G4], BF16, kind="ExternalInput")
    w1pT = nc.dram_tensor("w1pT", [4, 128, G4], BF16, kind="ExternalInput")
    b1T = nc.dram_tensor("b1T", [128, 16], F32, kind="ExternalInput")
    whh1T = nc.dram_tensor("whh1T", [4, 128, G4], WDT, kind="ExternalInput")
    pslot = nc.dram_tensor("pslot", [1, 1], mybir.dt.uint32, kind="ExternalInput")
    h1 = nc.dram_tensor("h1", [128, 32], BF16, kind="ExternalOutput")
    dbg_pre = nc.dram_tensor("dbg_pre", [128, 16, CH * BS], BF16, kind="ExternalOutput")
    dbg_loc = nc.dram_tensor("dbg_loc", [128, 4, 128], BF16, kind="ExternalOutput")

    with tile.TileContext(nc) as tc:
        with tc.tile_pool(name="dram", bufs=1, space="DRAM") as dram:
            locT = dram.tile([128, 4, TBP], BF16)
            revT = dram.tile([128, 4, TBP], BF16)
            ag = dram.tile([2, 128, 4, TBP], BF16)
            pbuf = dram.tile([128, 4, TBP], BF16)

            with (
                tc.tile_pool(name="outer", bufs=1) as op,
                tc.tile_pool(name="gpsum", bufs=1, space="PSUM") as gpsum,
                tc.tile_pool(name="ppsum", bufs=2, space="PSUM") as ppsum,
                tc.tile_pool(name="gtmp", bufs=2) as gp,
                tc.tile_pool(name="stage", bufs=4) as stp,
                tc.tile_pool(name="prebuf", bufs=1) as prep,
                tc.tile_pool(name="xin", bufs=2) as xp,
            ):
                bank_ig = gpsum.tile([128, 64], F32)
                bank_fo = gpsum.tile([128, 64], F32)
                cT = op.tile([128, 32], F32)
                hT = op.tile([128, 32], BF16)
                preA = prep.tile([128, 16, CH * BS], BF16)
                preB = prep.tile([128, 16, CH * BS], BF16)
                zlhs = op.tile([1, 128], BF16)
                zrhs = op.tile([1, 64], BF16)
                zslop = op.tile([128, 4, PAD], BF16)
                nc.gpsimd.memset(zlhs[:], 0.0)
                nc.gpsimd.memset(zrhs[:], 0.0)
                nc.gpsimd.memset(zslop[:], 0.0)
                nc.gpsimd.memset(cT[:], 0.0)
                nc.gpsimd.memset(hT[:], 0.0)

                # open the gate PSUM accumulation groups forever: write 0s with
                # start=True (sets has_written over the full [128, 64] region),
                # never issue stop.  All gate matmuls accumulate (start=False)
                # on top of DVE-prewritten pre values.
                nc.tensor.matmul(bank_ig[:], zlhs[:], zrhs[:],
                                 start=True, stop=False, skip_group_check=True)
                nc.tensor.matmul(bank_fo[:], zlhs[:], zrhs[:],
                                 start=True, stop=False, skip_group_check=True)

                # zero the lookahead slop so layer-1's projection reads are finite
                nc.sync.dma_start(locT[:, :, TB:TBP], zslop[:])
                nc.sync.dma_start(revT[:, :, TB:TBP], zslop[:])

                pools = dict(op=op, gp=gp, stp=stp, xp=xp, ppsum=ppsum,
                             bank_ig=bank_ig, bank_fo=bank_fo, cT=cT, hT=hT,
                             preA=preA, preB=preB)

                recur_layer(tc, pools, T, WDT,
                            xsrc=[(x_T, k) for k in range(4)],
                            wihT=[(w0T, k) for k in range(4)],
                            whhT=whh0T, biasT=b0T,
                            locT=locT, revT=revT, dbg_pre=dbg_pre)
                nc.sync.dma_start(dbg_loc[:], locT[:, :, 0:128])

                nc.gpsimd.collective_compute(
                    "AllGather",
                    mybir.AluOpType.bypass,
                    ins=[revT.opt()],
                    outs=[ag.opt()],
                    replica_groups=[[0, 1], [2, 3], [4, 5], [6, 7]],
                )

                # copy partner's gathered (pre-reversed) sequence to pbuf
                with tc.tile_pool(name="pslot", bufs=1) as pp:
                    pslot_sb = pp.tile([1, 1], mybir.dt.uint32)
                    nc.sync.dma_start(pslot_sb[:], pslot[:])
                    tmp_reg = nc.sync.alloc_register("pslot_reg")
                    nc.sync.reg_load(tmp_reg, pslot_sb[0:1, 0:1])
                    slot_reg = nc.sync.snap(tmp_reg, donate=True, min_val=0, max_val=1)
                    nc.sync.dma_start(
                        pbuf[:],
                        ag[ds(slot_reg, 1)].rearrange("s p k c -> (s p) k c"),
                    )

                nc.gpsimd.memset(cT[:], 0.0)
                nc.gpsimd.memset(hT[:], 0.0)

                recur_layer(tc, pools, T, WDT,
                            xsrc=[(locT, k) for k in range(4)]
                                 + [(pbuf, k) for k in range(4)],
                            wihT=[(w1oT, k) for k in range(4)]
                                 + [(w1pT, k) for k in range(4)],
                            whhT=whh1T, biasT=b1T)

                nc.sync.dma_start(h1[:], hT[:])

    nc.compile()
    return nc


def _xslice(src, k, col, n=128):
    """[128, n] moving slice at column `col` from an x-like source."""
    t, kk = src
    if len(t.shape) == 3 and t.shape[0] == 4:     # x_T [4, 128, TBP]
        return t[kk, :, col] if isinstance(col, slice) else t[kk, :, ds(col, n)]
    # locT/pbuf [128, 4, TBP]
    return t[:, kk, col] if isinstance(col, slice) else t[:, kk, ds(col, n)]


def recur_layer(tc, P, T, WDT, xsrc, wihT, whhT, biasT, locT=None, revT=None,
                dbg_pre=None):
    """One LSTM layer: fused projection + recurrence.

    xsrc: list of (dram_tensor, k) moving-operand sources, one per 128-wide
    contraction chunk (4 for layer 0, 4 own + 4 partner for layer 1).
    """
    nc = tc.nc
    nk = len(xsrc)
    store = locT is not None
    bank_ig, bank_fo = P["bank_ig"], P["bank_fo"]
    cT, hT, preA, preB = P["cT"], P["hT"], P["preA"], P["preB"]
    gp, stp, xp, ppsum, op = P["gp"], P["stp"], P["xp"], P["ppsum"], P["op"]
    REV = 8 * T - 8

    with tc.tile_pool(name="wpool", bufs=1) as wp:
        whh_sb = []
        for k in range(4):
            w = wp.tile([128, G4], WDT, name=f"whh{k}")
            nc.sync.dma_start(w[:], whhT[k])
            whh_sb.append(w)
        wih_sb = []
        for j, (t, kk) in enumerate(wihT):
            w = wp.tile([128, G4], BF16, name=f"wih{j}")
            nc.sync.dma_start(w[:], t[kk])
            wih_sb.append(w)
        bias_sb = wp.tile([128, 16], F32)
        nc.sync.dma_start(bias_sb[:], biasT[:])

        def dma_xchunk(parity, col):
            """Fetch the [128,128] moving tiles for one chunk's projection."""
            xts = []
            for j in range(nk):
                xt = xp.tile([128, 128], BF16, name=f"x{j}", tag=f"x{j}p{parity}")
                nc.sync.dma_start(xt[:], _xslice(xsrc[j], j, col))
                xts.append(xt)
            return xts

        def emit_proj(s, xts, dst, pps):
            """Projection matmuls for mslot s into pps col 128*(s%4)."""
            m = MORDER[s]
            col = 128 * (s % 4)
            for j in range(nk):
                nc.tensor.matmul(pps[:, col:col + 128],
                                 wih_sb[j][:, 128 * m:128 * (m + 1)], xts[j][:],
                                 start=(j == 0), stop=(j == nk - 1))

        def emit_proj_copies(g, dst, pps):
            """Evacuate mslots 4g..4g+3 (+bias) from pps into dst."""
            for jj in range(4):
                s = 4 * g + jj
                nc.vector.tensor_scalar_add(dst[:, s, :],
                                            pps[:, 128 * jj:128 * jj + 128],
                                            bias_sb[:, s:s + 1])

        def prewrite(nxt_s):
            """DVE-overwrite both gate banks with pre for step nxt_s."""
            buf = preA if (nxt_s // CH) % 2 == 0 else preB
            c0 = 8 * (nxt_s % CH)
            nc.vector.tensor_copy(
                bank_ig[:].rearrange("p (s b) -> p s b", s=8),
                buf[:, 0:8, c0:c0 + 8])
            nc.vector.tensor_copy(
                bank_fo[:].rearrange("p (s b) -> p s b", s=8),
                buf[:, 8:16, c0:c0 + 8])

        def emit_step(u, iv8, xts, dst, pps):
            """One LSTM step; consumes prewritten banks, prewrites step u+1.

            u: step index within the body (0..UNROLL-1); also emits the
            projection for mslot (u%CH) of the lookahead chunk into dst.
            """
            # gate matmuls: bank_ig = [i0..3 | g0..3]
            for j in range(4):
                for k in range(4):
                    nc.tensor.matmul(bank_ig[:, 8 * j:8 * j + 8],
                                     whh_sb[k][:, 128 * j:128 * (j + 1)],
                                     hT[:, 8 * k:8 * k + 8],
                                     start=False, stop=False, skip_group_check=True)
            for j in range(4):
                m = 8 + j
                for k in range(4):
                    nc.tensor.matmul(bank_ig[:, 32 + 8 * j:40 + 8 * j],
                                     whh_sb[k][:, 128 * m:128 * (m + 1)],
                                     hT[:, 8 * k:8 * k + 8],
                                     start=False, stop=False, skip_group_check=True)
            # bank_fo = [f0..3 | o0..3]
            for j in range(4):
                m = 4 + j
                for k in range(4):
                    nc.tensor.matmul(bank_fo[:, 8 * j:8 * j + 8],
                                     whh_sb[k][:, 128 * m:128 * (m + 1)],
                                     hT[:, 8 * k:8 * k + 8],
                                     start=False, stop=False, skip_group_check=True)
            for j in range(4):
                m = 12 + j
                for k in range(4):
                    nc.tensor.matmul(bank_fo[:, 32 + 8 * j:40 + 8 * j],
                                     whh_sb[k][:, 128 * m:128 * (m + 1)],
                                     hT[:, 8 * k:8 * k + 8],
                                     start=False, stop=False, skip_group_check=True)
            # projection filler (fills the PE while the gate chain runs)
            emit_proj(u % CH, xts, dst, pps)

            # gate chain
            tg = gp.tile([128, 32], F32, tag="tg")
            nc.scalar.activation(tg[:], bank_ig[:, 32:64], AF.Tanh, scale=0.0625)
            si = gp.tile([128, 32], F32, tag="si")
            nc.scalar.activation(si[:], bank_ig[:, 0:32], AF.Sigmoid, scale=0.0625)
            sfo = gp.tile([128, 64], F32, tag="sfo")
            nc.scalar.activation(sfo[:], bank_fo[:], AF.Sigmoid, scale=0.0625)

            a = gp.tile([128, 32], F32, tag="a")
            nc.vector.tensor_tensor(a[:], si[:], tg[:], MULT)
            # prewrite_ig as early as possible (only needs si/tg reads done)
            buf = preA if ((u + 1) // CH) % 2 == 0 else preB
            c0 = 8 * ((u + 1) % CH)
            nc.vector.tensor_copy(
                bank_ig[:].rearrange("p (s b) -> p s b", s=8),
                buf[:, 0:8, c0:c0 + 8])
            btmp = gp.tile([128, 32], F32, tag="btmp")
            nc.vector.tensor_tensor(btmp[:], sfo[:, 0:32], cT[:], MULT)
            nc.vector.tensor_tensor(cT[:], a[:], btmp[:], ADD)
            # evacuate the projection PSUM group every 4 mslots.  Must be
            # emitted before prewrite_fo: at u%CH==15 the prewrite reads pre
            # slots this group produces, and the DVE queue is strict FIFO.
            if u % 4 == 3:
                emit_proj_copies((u % CH) // 4, dst, pps)
            nc.vector.tensor_copy(
                bank_fo[:].rearrange("p (s b) -> p s b", s=8),
                buf[:, 8:16, c0:c0 + 8])
            tct = gp.tile([128, 32], F32, tag="tct")
            nc.scalar.activation(tct[:], cT[:], AF.Tanh)
            nc.vector.tensor_tensor(hT[:], sfo[:, 32:64], tct[:], MULT)

            if store:
                st = stp.tile([128, 32], BF16, tag="st")
                nc.gpsimd.tensor_copy(st[:], hT[:])
                st3 = st[:].rearrange("p (k b) -> p k b", k=4)
                nc.sync.dma_start(locT[:, :, ds(iv8, 8)], st3)
                nc.gpsimd.dma_start(revT[:, :, ds(REV - iv8, 8)], st3)

        # ---- prologue: produce chunk 0 into preA, prewrite step 0 ----
        xts0 = dma_xchunk(0, slice(0, 128))
        for g in range(4):
            pps = ppsum.tile([128, 512], F32, tag="pj", name="pps")
            for jj in range(4):
                emit_proj(4 * g + jj, xts0, preA, pps)
            emit_proj_copies(g, preA, pps)
        prewrite(0)
        if dbg_pre is not None:
            nc.sync.dma_start(dbg_pre[:], preA[:])

        # ---- main loop: 32 steps (2 chunks) per body ----
        with tc.For_i(0, 8 * T, 8 * UNROLL,
                      hint_engines=(mybir.EngineType.PE,)) as iv0:
            # lookahead chunk DMAs: first half produces chunk c+1 (parity 1),
            # second half produces chunk c+2 (parity 0)
            xts1 = dma_xchunk(1, iv0 + 8 * CH)
            xts2 = dma_xchunk(0, iv0 + 16 * CH)
            pps = None
            for u in range(UNROLL):
                if u % 4 == 0:
                    pps = ppsum.tile([128, 512], F32, tag="pj", name="pps")
                xts, dst = (xts1, preB) if u < CH else (xts2, preA)
                emit_step(u, iv0 + 8 * u, xts, dst, pps)


# ----------------------------------------------------------------------------
# Host side
# ----------------------------------------------------------------------------

_PROG_CACHE = {}


def _get_program(T):
    if T not in _PROG_CACHE:
        _PROG_CACHE[T] = build_program(T)
    return _PROG_CACHE[T]


def _bf16(a):
    import ml_dtypes
    return np.asarray(a, np.float32).astype(ml_dtypes.bfloat16)


def _fp8(a):
    import ml_dtypes
    return np.asarray(a, np.float32).astype(ml_dtypes.float8_e4m3)


SCALE = 16.0


def _chunkT(w, dtype_fn=_bf16, scale=SCALE):
    """[G4, K] weight -> scaled W^T [K//128, 128, G4]."""
    wt = np.ascontiguousarray(w.T.astype(np.float32)) * scale
    return dtype_fn(wt.reshape(wt.shape[0] // 128, 128, w.shape[0]))


def _biasT(b_ih, b_hh, scale=SCALE):
    """-> [128, 16] f32 per-mslot per-partition bias (x scale)."""
    b = (np.asarray(b_ih, np.float32) + np.asarray(b_hh, np.float32)) * scale
    out = np.zeros((128, 16), np.float32)
    for s, m in enumerate(MORDER):
        out[:, s] = b[128 * m:128 * (m + 1)]
    return out


def _prep_inputs(x, w_ih_f0, w_hh_f0, b_ih_f0, b_hh_f0,
                 w_ih_b0, w_hh_b0, b_ih_b0, b_hh_b0,
                 w_ih_f1, w_hh_f1, b_ih_f1, b_hh_f1,
                 w_ih_b1, w_hh_b1, b_ih_b1, b_hh_b1,
                 mask, T):
    f32 = np.float32
    wq = _fp8 if USE_FP8 else _bf16
    whh_f0m = (w_hh_f0 * mask).astype(f32)

    per_dir = {
        0: dict(w0T=_chunkT(w_ih_f0), b0T=_biasT(b_ih_f0, b_hh_f0),
                whh0T=_chunkT(whh_f0m, wq),
                w1oT=_chunkT(w_ih_f1[:, :H]), w1pT=_chunkT(w_ih_f1[:, H:]),
                b1T=_biasT(b_ih_f1, b_hh_f1),
                whh1T=_chunkT(w_hh_f1, wq)),
        1: dict(w0T=_chunkT(w_ih_b0), b0T=_biasT(b_ih_b0, b_hh_b0),
                whh0T=_chunkT(w_hh_b0, wq),
                w1oT=_chunkT(w_ih_b1[:, H:]), w1pT=_chunkT(w_ih_b1[:, :H]),
                b1T=_biasT(b_ih_b1, b_hh_b1),
                whh1T=_chunkT(w_hh_b1, wq)),
    }

    TB = T * BS
    in_maps = []
    for core in range(NCORES):
        pair, q = core // 2, core % 2
        xs = x[pair * BS:(pair + 1) * BS, :T].astype(f32)   # [8, T, E]
        if q == 1:
            xs = xs[:, ::-1]
        xT = np.ascontiguousarray(xs.transpose(2, 1, 0)).reshape(4, 128, TB)
        xTp = np.zeros((4, 128, TB + PAD), np.float32)
        xTp[:, :, :TB] = xT
        m = dict(per_dir[q])
        m["x_T"] = _bf16(xTp)
        m["pslot"] = np.array([[1 - q]], dtype=np.uint32)
        in_maps.append(m)
    return in_maps


def _mish(x):
    return x * np.tanh(np.log1p(np.exp(-np.abs(x))) + np.maximum(x, 0.0))


def _unT(hT):
    """[128, 32] h^T-chunk layout -> [8, 512]: h[b, 128k+p] = hT[p, 8k+b]."""
    a = np.asarray(hT, np.float32).reshape(128, 4, 8)     # p, k, b
    return np.ascontiguousarray(a.transpose(2, 1, 0)).reshape(8, 512)


def _head(h1s, fc_w, fc_b):
    h1s = [_unT(h) for h in h1s]
    h_f = np.concatenate([np.asarray(h1s[2 * p], np.float32) for p in range(4)], axis=0)
    h_b = np.concatenate([np.asarray(h1s[2 * p + 1], np.float32) for p in range(4)], axis=0)
    h = 0.5 * (h_f + h_b)
    z = h @ np.asarray(fc_w, np.float32).T + np.asarray(fc_b, np.float32)
    return _mish(z).astype(np.float32)


def run_device(inputs, trace=False, tmpdir=None):
    from concourse.bass_utils import run_bass_kernel_spmd

    x = inputs["x"]
    T = x.shape[1]
    nc = _get_program(T)
    in_maps = _prep_inputs(
        x, inputs["w_ih_f0"], inputs["w_hh_f0"], inputs["b_ih_f0"], inputs["b_hh_f0"],
        inputs["w_ih_b0"], inputs["w_hh_b0"], inputs["b_ih_b0"], inputs["b_hh_b0"],
        inputs["w_ih_f1"], inputs["w_hh_f1"], inputs["b_ih_f1"], inputs["b_hh_f1"],
        inputs["w_ih_b1"], inputs["w_hh_b1"], inputs["b_ih_b1"], inputs["b_hh_b1"],
        inputs["mask"], T)

    res = run_bass_kernel_spmd(nc, in_maps, list(range(NCORES)),
                               trace=trace, tmpdir=tmpdir)
    h1s = [res.results[c]["h1"] for c in range(NCORES)]
    return h1s, res


def kernel(**inputs):
    h1s, _ = run_device(inputs)
    return _head(h1s, inputs["fc_w"], inputs["fc_b"])


# revision 13
# speedup vs baseline: 1.7167x; 1.0018x over previous
"""AWD-LSTM forward on 8 Trainium2 NeuronCores — v2 "warm" design.

Sharding (unchanged from v1): 8 cores = 4 pairs; even core of a pair runs
the forward direction, odd the backward (fed time-reversed inputs so both
are forward scans).  Batch (32) is sharded 8 rows per pair.  Layer-1 input
needs both directions' layer-0 sequences, exchanged with a pairwise
AllGather.  The tiny fc+mish head runs on host.

v2 changes (vs the 16.3 ms v1):
  - The input projections (pre = x @ W_ih^T + bias) are FUSED into the
    recurrence loop: each step's tail (the ACT/DVE gate chain, ~1.5 us of
    PE idle in v1) is filled with a few projection matmuls that compute
    pre for steps ~16-32 ahead, into SBUF ping-pong chunk buffers.  This
    keeps the PE busy so the HAM clock gate stays at K=8/8 (2.4 GHz) —
    v1 ran throttled at 1.2 GHz for 94% of the kernel — and absorbs the
    ~0.7 ms standalone projection phases.
  - The 16 per-step pre-fold transpose-matmuls are gone: pre enters the
    gate PSUM banks via a DVE value-overwrite while the accumulation
    group stays open forever (one dummy start=True matmul at init; all
    gate matmuls start=False).  has_written bits stay set, so matmuls
    accumulate on top of the DVE-written pre values.
  - W_hh is fp8-e4m3 (stationary side only; h stays bf16): FWL loads
    fp8 weights 2x faster than bf16.  Everything is pre-scaled by 16 on
    host (clears e4m3's subnormal floor) and descaled for free via the
    ACT affine (scale=1/16).  Measured fp8 impact on final error vs
    fp32 reference: 6.7e-3 (budget 2e-2).
  - Gate banks laid out [i|g] and [f|o] so the exposed tail chain is
    sigmoid(f,o) -> f*c -> +i*tanh(g) -> tanh(c) -> o*tanh(c), with the
    i/g activations hidden under the f/o matmuls.
"""

import sys

sys.path.insert(0, "/opt/trn_rl_repo")

import numpy as np

import concourse.bass as bass
import concourse.bacc as bacc
import concourse.mybir as mybir
import concourse.tile as tile
from concourse.bass import ds

B, S, E = 32, 1024, 512
H = E
G4 = 4 * H            # 2048
BS = 8                # batch rows per pair
NCORES = 8
CH = 16               # steps per pre chunk
UNROLL = 32           # steps per loop body (2 chunks)
PAD = 2 * CH * BS     # x/seq column padding (2 chunks of lookahead slop)
F32 = mybir.dt.float32
BF16 = mybir.dt.bfloat16
FP8 = mybir.dt.float8e4
USE_FP8 = True

AF = mybir.ActivationFunctionType
MULT = mybir.AluOpType.mult
ADD = mybir.AluOpType.add

# mslot s (production/bias/pre-buffer order) -> real m-chunk (gate dim / 128)
# slots 0-7 -> bank_ig [i0..3, g0..3]; slots 8-15 -> bank_fo [f0..3, o0..3]
MORDER = [0, 1, 2, 3, 8, 9, 10, 11, 4, 5, 6, 7, 12, 13, 14, 15]


def build_program(T=S, fp8=USE_FP8):
    TB = T * BS
    TBP = TB + PAD
    WDT = FP8 if fp8 else BF16
    nc = bacc.Bacc(None, target_bir_lowering=False)

    # ---- I/O ----
    x_T = nc.dram_tensor("x_T", [4, 128, TBP], BF16, kind="ExternalInput")
    w0T = nc.dram_tensor("w0T", [4, 128, G4], BF16, kind="ExternalInput")
    b0T = nc.dram_tensor("b0T", [128, 16], F32, kind="ExternalInput")
    whh0T = nc.dram_tensor("whh0T", [4, 128, G4], WDT, kind="ExternalInput")
    w1oT = nc.dram_tensor("w1oT", [4, 128, G4], BF16, kind="ExternalInput")
    w1pT = nc.dram_tensor("w1pT", [4, 128, G4], BF16, kind="ExternalInput")
    b1T = nc.dram_tensor("b1T", [128, 16], F32, kind="ExternalInput")
    whh1T = nc.dram_tensor("whh1T", [4, 128, G4], WDT, kind="ExternalInput")
    pslot = nc.dram_tensor("pslot", [1, 1], mybir.dt.uint32, kind="ExternalInput")
    h1 = nc.dram_tensor("h1", [128, 32], BF16, kind="ExternalOutput")
    dbg_pre = nc.dram_tensor("dbg_pre", [128, 16, CH * BS], BF16, kind="ExternalOutput")
    dbg_loc = nc.dram_tensor("dbg_loc", [128, 4, 128], BF16, kind="ExternalOutput")

    with tile.TileContext(nc) as tc:
        with tc.tile_pool(name="dram", bufs=1, space="DRAM") as dram:
            locT = dram.tile([128, 4, TBP], BF16)
            revT = dram.tile([128, 4, TBP], BF16)
            ag = dram.tile([2, 128, 4, TBP], BF16)
            pbuf = dram.tile([128, 4, TBP], BF16)

            with (
                tc.tile_pool(name="outer", bufs=1) as op,
                tc.tile_pool(name="gpsum", bufs=1, space="PSUM") as gpsum,
                tc.tile_pool(name="ppsum", bufs=2, space="PSUM") as ppsum,
                tc.tile_pool(name="gtmp", bufs=2) as gp,
                tc.tile_pool(name="stage", bufs=4) as stp,
                tc.tile_pool(name="prebuf", bufs=1) as prep,
                tc.tile_pool(name="xin", bufs=2) as xp,
            ):
                bank_ig = gpsum.tile([128, 64], F32)
                bank_fo = gpsum.tile([128, 64], F32)
                cT = op.tile([128, 32], F32)
                hT = op.tile([128, 32], BF16)
                preA = prep.tile([128, 16, CH * BS], BF16)
                preB = prep.tile([128, 16, CH * BS], BF16)
                zlhs = op.tile([1, 128], BF16)
                zrhs = op.tile([1, 64], BF16)
                zslop = op.tile([128, 4, PAD], BF16)
                nc.gpsimd.memset(zlhs[:], 0.0)
                nc.gpsimd.memset(zrhs[:], 0.0)
                nc.gpsimd.memset(zslop[:], 0.0)
                nc.gpsimd.memset(cT[:], 0.0)
                nc.gpsimd.memset(hT[:], 0.0)

                # open the gate PSUM accumulation groups forever: write 0s with
                # start=True (sets has_written over the full [128, 64] region),
                # never issue stop.  All gate matmuls accumulate (start=False)
                # on top of DVE-prewritten pre values.
                nc.tensor.matmul(bank_ig[:], zlhs[:], zrhs[:],
                                 start=True, stop=False, skip_group_check=True)
                nc.tensor.matmul(bank_fo[:], zlhs[:], zrhs[:],
                                 start=True, stop=False, skip_group_check=True)

                # zero the lookahead slop so layer-1's projection reads are finite
                nc.sync.dma_start(locT[:, :, TB:TBP], zslop[:])
                nc.sync.dma_start(revT[:, :, TB:TBP], zslop[:])

                pools = dict(op=op, gp=gp, stp=stp, xp=xp, ppsum=ppsum,
                             bank_ig=bank_ig, bank_fo=bank_fo, cT=cT, hT=hT,
                             preA=preA, preB=preB)

                recur_layer(tc, pools, T, WDT,
                            xsrc=[(x_T, k) for k in range(4)],
                            wihT=[(w0T, k) for k in range(4)],
                            whhT=whh0T, biasT=b0T,
                            locT=locT, revT=revT, dbg_pre=dbg_pre)
                nc.sync.dma_start(dbg_loc[:], locT[:, :, 0:128])

                nc.gpsimd.collective_compute(
                    "AllGather",
                    mybir.AluOpType.bypass,
                    ins=[revT.opt()],
                    outs=[ag.opt()],
                    replica_groups=[[0, 1], [2, 3], [4, 5], [6, 7]],
                )

                # copy partner's gathered (pre-reversed) sequence to pbuf
                with tc.tile_pool(name="pslot", bufs=1) as pp:
                    pslot_sb = pp.tile([1, 1], mybir.dt.uint32)
                    nc.sync.dma_start(pslot_sb[:], pslot[:])
                    tmp_reg = nc.sync.alloc_register("pslot_reg")
                    nc.sync.reg_load(tmp_reg, pslot_sb[0:1, 0:1])
                    slot_reg = nc.sync.snap(tmp_reg, donate=True, min_val=0, max_val=1)
                    nc.sync.dma_start(
                        pbuf[:],
                        ag[ds(slot_reg, 1)].rearrange("s p k c -> (s p) k c"),
                    )

                nc.gpsimd.memset(cT[:], 0.0)
                nc.gpsimd.memset(hT[:], 0.0)

                recur_layer(tc, pools, T, WDT,
                            xsrc=[(locT, k) for k in range(4)]
                                 + [(pbuf, k) for k in range(4)],
                            wihT=[(w1oT, k) for k in range(4)]
                                 + [(w1pT, k) for k in range(4)],
                            whhT=whh1T, biasT=b1T)

                nc.sync.dma_start(h1[:], hT[:])

    nc.compile()
    return nc


def _xslice(src, k, col, n=128):
    """[128, n] moving slice at column `col` from an x-like source."""
    t, kk = src
    if len(t.shape) == 3 and t.shape[0] == 4:     # x_T [4, 128, TBP]
        return t[kk, :, col] if isinstance(col, slice) else t[kk, :, ds(col, n)]
    # locT/pbuf [128, 4, TBP]
    return t[:, kk, col] if isinstance(col, slice) else t[:, kk, ds(col, n)]


def recur_layer(tc, P, T, WDT, xsrc, wihT, whhT, biasT, locT=None, revT=None,
                dbg_pre=None):
    """One LSTM layer: fused projection + recurrence.

    xsrc: list of (dram_tensor, k) moving-operand sources, one per 128-wide
    contraction chunk (4 for layer 0, 4 own + 4 partner for layer 1).
    """
    nc = tc.nc
    nk = len(xsrc)
    store = locT is not None
    bank_ig, bank_fo = P["bank_ig"], P["bank_fo"]
    cT, hT, preA, preB = P["cT"], P["hT"], P["preA"], P["preB"]
    gp, stp, xp, ppsum, op = P["gp"], P["stp"], P["xp"], P["ppsum"], P["op"]
    REV = 8 * T - 8

    with tc.tile_pool(name="wpool", bufs=1) as wp:
        whh_sb = []
        for k in range(4):
            w = wp.tile([128, G4], WDT, name=f"whh{k}")
            nc.sync.dma_start(w[:], whhT[k])
            whh_sb.append(w)
        wih_sb = []
        for j, (t, kk) in enumerate(wihT):
            w = wp.tile([128, G4], BF16, name=f"wih{j}")
            nc.sync.dma_start(w[:], t[kk])
            wih_sb.append(w)
        bias_sb = wp.tile([128, 16], F32)
        nc.sync.dma_start(bias_sb[:], biasT[:])

        def dma_xchunk(parity, col):
            """Fetch the [128,128] moving tiles for one chunk's projection."""
            xts = []
            for j in range(nk):
                xt = xp.tile([128, 128], BF16, name=f"x{j}", tag=f"x{j}p{parity}")
                nc.sync.dma_start(xt[:], _xslice(xsrc[j], j, col))
                xts.append(xt)
            return xts

        def emit_proj(s, xts, dst, pps):
            """Projection matmuls for mslot s into pps col 128*(s%4)."""
            m = MORDER[s]
            col = 128 * (s % 4)
            for j in range(nk):
                nc.tensor.matmul(pps[:, col:col + 128],
                                 wih_sb[j][:, 128 * m:128 * (m + 1)], xts[j][:],
                                 start=(j == 0), stop=(j == nk - 1))

        def emit_proj_copies(g, dst, pps):
            """Evacuate mslots 4g..4g+3 (+bias) from pps into dst."""
            for jj in range(4):
                s = 4 * g + jj
                nc.vector.tensor_scalar_add(dst[:, s, :],
                                            pps[:, 128 * jj:128 * jj + 128],
                                            bias_sb[:, s:s + 1])

        def prewrite(nxt_s):
            """DVE-overwrite both gate banks with pre for step nxt_s."""
            buf = preA if (nxt_s // CH) % 2 == 0 else preB
            c0 = 8 * (nxt_s % CH)
            nc.vector.tensor_copy(
                bank_ig[:].rearrange("p (s b) -> p s b", s=8),
                buf[:, 0:8, c0:c0 + 8])
            nc.vector.tensor_copy(
                bank_fo[:].rearrange("p (s b) -> p s b", s=8),
                buf[:, 8:16, c0:c0 + 8])

        def emit_step(u, iv8, xts, dst, pps):
            """One LSTM step; consumes prewritten banks, prewrites step u+1.

            u: step index within the body (0..UNROLL-1); also emits the
            projection for mslot (u%CH) of the lookahead chunk into dst.
            """
            # gate matmuls: bank_ig = [i0..3 | g0..3]
            for j in range(4):
                for k in range(4):
                    nc.tensor.matmul(bank_ig[:, 8 * j:8 * j + 8],
                                     whh_sb[k][:, 128 * j:128 * (j + 1)],
                                     hT[:, 8 * k:8 * k + 8],
                                     start=False, stop=False, skip_group_check=True)
            for j in range(4):
                m = 8 + j
                for k in range(4):
                    nc.tensor.matmul(bank_ig[:, 32 + 8 * j:40 + 8 * j],
                                     whh_sb[k][:, 128 * m:128 * (m + 1)],
                                     hT[:, 8 * k:8 * k + 8],
                                     start=False, stop=False, skip_group_check=True)
            # emit the ig activations BEFORE the fo matmuls: the PE->ACT
            # semaphore inc then lands right after the last ig matmul, so
            # tanh(g)/sig(i)/a run UNDER the fo matmul stream.
            tg = gp.tile([128, 32], F32, tag="tg")
            nc.scalar.activation(tg[:], bank_ig[:, 32:64], AF.Tanh, scale=0.0625)
            si = gp.tile([128, 32], F32, tag="si")
            nc.scalar.activation(si[:], bank_ig[:, 0:32], AF.Sigmoid, scale=0.0625)

            # bank_fo = [f0..3 | o0..3]
            for j in range(4):
                m = 4 + j
                for k in range(4):
                    nc.tensor.matmul(bank_fo[:, 8 * j:8 * j + 8],
                                     whh_sb[k][:, 128 * m:128 * (m + 1)],
                                     hT[:, 8 * k:8 * k + 8],
                                     start=False, stop=False, skip_group_check=True)
            for j in range(4):
                m = 12 + j
                for k in range(4):
                    nc.tensor.matmul(bank_fo[:, 32 + 8 * j:40 + 8 * j],
                                     whh_sb[k][:, 128 * m:128 * (m + 1)],
                                     hT[:, 8 * k:8 * k + 8],
                                     start=False, stop=False, skip_group_check=True)
            sfo = gp.tile([128, 64], F32, tag="sfo")
            nc.scalar.activation(sfo[:], bank_fo[:], AF.Sigmoid, scale=0.0625)

            a = gp.tile([128, 32], F32, tag="a")
            nc.vector.tensor_tensor(a[:], si[:], tg[:], MULT)
            # prewrite_ig as early as possible (only needs si/tg reads done)
            buf = preA if ((u + 1) // CH) % 2 == 0 else preB
            c0 = 8 * ((u + 1) % CH)
            nc.vector.tensor_copy(
                bank_ig[:].rearrange("p (s b) -> p s b", s=8),
                buf[:, 0:8, c0:c0 + 8])
            btmp = gp.tile([128, 32], F32, tag="btmp")
            nc.vector.tensor_tensor(btmp[:], sfo[:, 0:32], cT[:], MULT)
            nc.vector.tensor_tensor(cT[:], a[:], btmp[:], ADD)
            # projection filler: emitted after the gate ACTs (so their PE
            # semaphore incs land right after each bank's last matmul) but
            # before the proj copies (program-order dependency tracking).
            emit_proj(u % CH, xts, dst, pps)
            # evacuate the projection PSUM group every 4 mslots.  Must be
            # emitted before prewrite_fo: at u%CH==15 the prewrite reads pre
            # slots this group produces, and the DVE queue is strict FIFO.
            if u % 4 == 3:
                emit_proj_copies((u % CH) // 4, dst, pps)
            nc.vector.tensor_copy(
                bank_fo[:].rearrange("p (s b) -> p s b", s=8),
                buf[:, 8:16, c0:c0 + 8])
            tct = gp.tile([128, 32], F32, tag="tct")
            nc.scalar.activation(tct[:], cT[:], AF.Tanh)
            nc.vector.tensor_tensor(hT[:], sfo[:, 32:64], tct[:], MULT)

            if store:
                st = stp.tile([128, 32], BF16, tag="st")
                nc.gpsimd.tensor_copy(st[:], hT[:])
                st3 = st[:].rearrange("p (k b) -> p k b", k=4)
                nc.sync.dma_start(locT[:, :, ds(iv8, 8)], st3)
                nc.gpsimd.dma_start(revT[:, :, ds(REV - iv8, 8)], st3)

        # ---- prologue: produce chunk 0 into preA, prewrite step 0 ----
        xts0 = dma_xchunk(0, slice(0, 128))
        for g in range(4):
            pps = ppsum.tile([128, 512], F32, tag="pj", name="pps")
            for jj in range(4):
                emit_proj(4 * g + jj, xts0, preA, pps)
            emit_proj_copies(g, preA, pps)
        prewrite(0)
        if dbg_pre is not None:
            nc.sync.dma_start(dbg_pre[:], preA[:])

        # ---- main loop: 32 steps (2 chunks) per body ----
        with tc.For_i(0, 8 * T, 8 * UNROLL,
                      hint_engines=(mybir.EngineType.PE,)) as iv0:
            # lookahead chunk DMAs: first half produces chunk c+1 (parity 1),
            # second half produces chunk c+2 (parity 0)
            xts1 = dma_xchunk(1, iv0 + 8 * CH)
            xts2 = dma_xchunk(0, iv0 + 16 * CH)
            pps = None
            for u in range(UNROLL):
                if u % 4 == 0:
                    pps = ppsum.tile([128, 512], F32, tag="pj", name="pps")
                xts, dst = (xts1, preB) if u < CH else (xts2, preA)
                emit_step(u, iv0 + 8 * u, xts, dst, pps)


# ----------------------------------------------------------------------------
# Host side
# ----------------------------------------------------------------------------

_PROG_CACHE = {}


def _get_program(T):
    if T not in _PROG_CACHE:
        _PROG_CACHE[T] = build_program(T)
    return _PROG_CACHE[T]


def _bf16(a):
    import ml_dtypes
    return np.asarray(a, np.float32).astype(ml_dtypes.bfloat16)


def _fp8(a):
    import ml_dtypes
    return np.asarray(a, np.float32).astype(ml_dtypes.float8_e4m3)


SCALE = 16.0


def _chunkT(w, dtype_fn=_bf16, scale=SCALE):
    """[G4, K] weight -> scaled W^T [K//128, 128, G4]."""
    wt = np.ascontiguousarray(w.T.astype(np.float32)) * scale
    return dtype_fn(wt.reshape(wt.shape[0] // 128, 128, w.shape[0]))


def _biasT(b_ih, b_hh, scale=SCALE):
    """-> [128, 16] f32 per-mslot per-partition bias (x scale)."""
    b = (np.asarray(b_ih, np.float32) + np.asarray(b_hh, np.float32)) * scale
    out = np.zeros((128, 16), np.float32)
    for s, m in enumerate(MORDER):
        out[:, s] = b[128 * m:128 * (m + 1)]
    return out


def _prep_inputs(x, w_ih_f0, w_hh_f0, b_ih_f0, b_hh_f0,
                 w_ih_b0, w_hh_b0, b_ih_b0, b_hh_b0,
                 w_ih_f1, w_hh_f1, b_ih_f1, b_hh_f1,
                 w_ih_b1, w_hh_b1, b_ih_b1, b_hh_b1,
                 mask, T):
    f32 = np.float32
    wq = _fp8 if USE_FP8 else _bf16
    whh_f0m = (w_hh_f0 * mask).astype(f32)

    per_dir = {
        0: dict(w0T=_chunkT(w_ih_f0), b0T=_biasT(b_ih_f0, b_hh_f0),
                whh0T=_chunkT(whh_f0m, wq),
                w1oT=_chunkT(w_ih_f1[:, :H]), w1pT=_chunkT(w_ih_f1[:, H:]),
                b1T=_biasT(b_ih_f1, b_hh_f1),
                whh1T=_chunkT(w_hh_f1, wq)),
        1: dict(w0T=_chunkT(w_ih_b0), b0T=_biasT(b_ih_b0, b_hh_b0),
                whh0T=_chunkT(w_hh_b0, wq),
                w1oT=_chunkT(w_ih_b1[:, H:]), w1pT=_chunkT(w_ih_b1[:, :H]),
                b1T=_biasT(b_ih_b1, b_hh_b1),
                whh1T=_chunkT(w_hh_b1, wq)),
    }

    TB = T * BS
    in_maps = []
    for core in range(NCORES):
        pair, q = core // 2, core % 2
        xs = x[pair * BS:(pair + 1) * BS, :T].astype(f32)   # [8, T, E]
        if q == 1:
            xs = xs[:, ::-1]
        xT = np.ascontiguousarray(xs.transpose(2, 1, 0)).reshape(4, 128, TB)
        xTp = np.zeros((4, 128, TB + PAD), np.float32)
        xTp[:, :, :TB] = xT
        m = dict(per_dir[q])
        m["x_T"] = _bf16(xTp)
        m["pslot"] = np.array([[1 - q]], dtype=np.uint32)
        in_maps.append(m)
    return in_maps


def _mish(x):
    return x * np.tanh(np.log1p(np.exp(-np.abs(x))) + np.maximum(x, 0.0))


def _unT(hT):
    """[128, 32] h^T-chunk layout -> [8, 512]: h[b, 128k+p] = hT[p, 8k+b]."""
    a = np.asarray(hT, np.float32).reshape(128, 4, 8)     # p, k, b
    return np.ascontiguousarray(a.transpose(2, 1, 0)).reshape(8, 512)


def _head(h1s, fc_w, fc_b):
    h1s = [_unT(h) for h in h1s]
    h_f = np.concatenate([np.asarray(h1s[2 * p], np.float32) for p in range(4)], axis=0)
    h_b = np.concatenate([np.asarray(h1s[2 * p + 1], np.float32) for p in range(4)], axis=0)
    h = 0.5 * (h_f + h_b)
    z = h @ np.asarray(fc_w, np.float32).T + np.asarray(fc_b, np.float32)
    return _mish(z).astype(np.float32)


def run_device(inputs, trace=False, tmpdir=None):
    from concourse.bass_utils import run_bass_kernel_spmd

    x = inputs["x"]
    T = x.shape[1]
    nc = _get_program(T)
    in_maps = _prep_inputs(
        x, inputs["w_ih_f0"], inputs["w_hh_f0"], inputs["b_ih_f0"], inputs["b_hh_f0"],
        inputs["w_ih_b0"], inputs["w_hh_b0"], inputs["b_ih_b0"], inputs["b_hh_b0"],
        inputs["w_ih_f1"], inputs["w_hh_f1"], inputs["b_ih_f1"], inputs["b_hh_f1"],
        inputs["w_ih_b1"], inputs["w_hh_b1"], inputs["b_ih_b1"], inputs["b_hh_b1"],
        inputs["mask"], T)

    res = run_bass_kernel_spmd(nc, in_maps, list(range(NCORES)),
                               trace=trace, tmpdir=tmpdir)
    h1s = [res.results[c]["h1"] for c in range(NCORES)]
    return h1s, res


def kernel(**inputs):
    h1s, _ = run_device(inputs)
    return _head(h1s, inputs["fc_w"], inputs["fc_b"])


# revision 16
# speedup vs baseline: 1.7852x; 1.0399x over previous
"""AWD-LSTM forward on 8 Trainium2 NeuronCores — v2 "warm" design.

Sharding (unchanged from v1): 8 cores = 4 pairs; even core of a pair runs
the forward direction, odd the backward (fed time-reversed inputs so both
are forward scans).  Batch (32) is sharded 8 rows per pair.  Layer-1 input
needs both directions' layer-0 sequences, exchanged with a pairwise
AllGather.  The tiny fc+mish head runs on host.

v2 changes (vs the 16.3 ms v1):
  - The input projections (pre = x @ W_ih^T + bias) are FUSED into the
    recurrence loop: each step's tail (the ACT/DVE gate chain, ~1.5 us of
    PE idle in v1) is filled with a few projection matmuls that compute
    pre for steps ~16-32 ahead, into SBUF ping-pong chunk buffers.  This
    keeps the PE busy so the HAM clock gate stays at K=8/8 (2.4 GHz) —
    v1 ran throttled at 1.2 GHz for 94% of the kernel — and absorbs the
    ~0.7 ms standalone projection phases.
  - The 16 per-step pre-fold transpose-matmuls are gone: pre enters the
    gate PSUM banks via a DVE value-overwrite while the accumulation
    group stays open forever (one dummy start=True matmul at init; all
    gate matmuls start=False).  has_written bits stay set, so matmuls
    accumulate on top of the DVE-written pre values.
  - W_hh is fp8-e4m3 (stationary side only; h stays bf16): FWL loads
    fp8 weights 2x faster than bf16.  Everything is pre-scaled by 16 on
    host (clears e4m3's subnormal floor) and descaled for free via the
    ACT affine (scale=1/16).  Measured fp8 impact on final error vs
    fp32 reference: 6.7e-3 (budget 2e-2).
  - Gate banks laid out [i|g] and [f|o] so the exposed tail chain is
    sigmoid(f,o) -> f*c -> +i*tanh(g) -> tanh(c) -> o*tanh(c), with the
    i/g activations hidden under the f/o matmuls.
"""

import sys

sys.path.insert(0, "/opt/trn_rl_repo")

import numpy as np

import concourse.bass as bass
import concourse.bacc as bacc
import concourse.mybir as mybir
import concourse.tile as tile
from concourse.bass import ds

B, S, E = 32, 1024, 512
H = E
G4 = 4 * H            # 2048
BS = 8                # batch rows per pair
NCORES = 8
CH = 16               # steps per pre chunk
UNROLL = 64           # steps per loop body (4 chunks)
PAD = 2 * CH * BS     # x/seq column padding (2 chunks of lookahead slop)
F32 = mybir.dt.float32
BF16 = mybir.dt.bfloat16
FP8 = mybir.dt.float8e4
USE_FP8 = True

AF = mybir.ActivationFunctionType
MULT = mybir.AluOpType.mult
ADD = mybir.AluOpType.add

# mslot s (production/bias/pre-buffer order) -> real m-chunk (gate dim / 128)
# slots 0-7 -> bank_ig [i0..3, g0..3]; slots 8-15 -> bank_fo [f0..3, o0..3]
MORDER = [0, 1, 2, 3, 8, 9, 10, 11, 4, 5, 6, 7, 12, 13, 14, 15]


def build_program(T=S, fp8=USE_FP8):
    TB = T * BS
    TBP = TB + PAD
    WDT = FP8 if fp8 else BF16
    nc = bacc.Bacc(None, target_bir_lowering=False)

    # ---- I/O ----
    x_T = nc.dram_tensor("x_T", [4, 128, TBP], BF16, kind="ExternalInput")
    w0T = nc.dram_tensor("w0T", [4, 128, G4], BF16, kind="ExternalInput")
    b0T = nc.dram_tensor("b0T", [128, 16], F32, kind="ExternalInput")
    whh0T = nc.dram_tensor("whh0T", [4, 128, G4], WDT, kind="ExternalInput")
    w1oT = nc.dram_tensor("w1oT", [4, 128, G4], BF16, kind="ExternalInput")
    w1pT = nc.dram_tensor("w1pT", [4, 128, G4], BF16, kind="ExternalInput")
    b1T = nc.dram_tensor("b1T", [128, 16], F32, kind="ExternalInput")
    whh1T = nc.dram_tensor("whh1T", [4, 128, G4], WDT, kind="ExternalInput")
    pslot = nc.dram_tensor("pslot", [1, 1], mybir.dt.uint32, kind="ExternalInput")
    h1 = nc.dram_tensor("h1", [128, 32], BF16, kind="ExternalOutput")
    dbg_pre = nc.dram_tensor("dbg_pre", [128, 16, CH * BS], BF16, kind="ExternalOutput")
    dbg_loc = nc.dram_tensor("dbg_loc", [128, 4, 128], BF16, kind="ExternalOutput")

    with tile.TileContext(nc) as tc:
        with tc.tile_pool(name="dram", bufs=1, space="DRAM") as dram:
            locT = dram.tile([128, 4, TBP], BF16)
            revT = dram.tile([128, 4, TBP], BF16)
            ag = dram.tile([2, 128, 4, TBP], BF16)
            pbuf = dram.tile([128, 4, TBP], BF16)

            with (
                tc.tile_pool(name="outer", bufs=1) as op,
                tc.tile_pool(name="gpsum", bufs=1, space="PSUM") as gpsum,
                tc.tile_pool(name="ppsum", bufs=2, space="PSUM") as ppsum,
                tc.tile_pool(name="gtmp", bufs=2) as gp,
                tc.tile_pool(name="stage", bufs=4) as stp,
                tc.tile_pool(name="prebuf", bufs=1) as prep,
                tc.tile_pool(name="xin", bufs=2) as xp,
            ):
                bank_ig = gpsum.tile([128, 64], F32)
                bank_fo = gpsum.tile([128, 64], F32)
                cT = op.tile([128, 32], F32)
                hT = op.tile([128, 32], BF16)
                pres = [prep.tile([128, 16, CH * BS], BF16, name=f"pre{i}")
                        for i in range(4)]
                zlhs = op.tile([1, 128], BF16)
                zrhs = op.tile([1, 64], BF16)
                zslop = op.tile([128, 4, PAD], BF16)
                nc.gpsimd.memset(zlhs[:], 0.0)
                nc.gpsimd.memset(zrhs[:], 0.0)
                nc.gpsimd.memset(zslop[:], 0.0)
                nc.gpsimd.memset(cT[:], 0.0)
                nc.gpsimd.memset(hT[:], 0.0)

                # open the gate PSUM accumulation groups forever: write 0s with
                # start=True (sets has_written over the full [128, 64] region),
                # never issue stop.  All gate matmuls accumulate (start=False)
                # on top of DVE-prewritten pre values.
                nc.tensor.matmul(bank_ig[:], zlhs[:], zrhs[:],
                                 start=True, stop=False, skip_group_check=True)
                nc.tensor.matmul(bank_fo[:], zlhs[:], zrhs[:],
                                 start=True, stop=False, skip_group_check=True)

                # zero the lookahead slop so layer-1's projection reads are finite
                nc.sync.dma_start(locT[:, :, TB:TBP], zslop[:])
                nc.sync.dma_start(revT[:, :, TB:TBP], zslop[:])

                pools = dict(op=op, gp=gp, stp=stp, xp=xp, ppsum=ppsum,
                             bank_ig=bank_ig, bank_fo=bank_fo, cT=cT, hT=hT,
                             pres=pres)

                recur_layer(tc, pools, T, WDT,
                            xsrc=[(x_T, k) for k in range(4)],
                            wihT=[(w0T, k) for k in range(4)],
                            whhT=whh0T, biasT=b0T,
                            locT=locT, revT=revT, dbg_pre=dbg_pre)
                nc.sync.dma_start(dbg_loc[:], locT[:, :, 0:128])

                nc.gpsimd.collective_compute(
                    "AllGather",
                    mybir.AluOpType.bypass,
                    ins=[revT.opt()],
                    outs=[ag.opt()],
                    replica_groups=[[0, 1], [2, 3], [4, 5], [6, 7]],
                )

                # copy partner's gathered (pre-reversed) sequence to pbuf
                with tc.tile_pool(name="pslot", bufs=1) as pp:
                    pslot_sb = pp.tile([1, 1], mybir.dt.uint32)
                    nc.sync.dma_start(pslot_sb[:], pslot[:])
                    tmp_reg = nc.sync.alloc_register("pslot_reg")
                    nc.sync.reg_load(tmp_reg, pslot_sb[0:1, 0:1])
                    slot_reg = nc.sync.snap(tmp_reg, donate=True, min_val=0, max_val=1)
                    nc.sync.dma_start(
                        pbuf[:],
                        ag[ds(slot_reg, 1)].rearrange("s p k c -> (s p) k c"),
                    )

                nc.gpsimd.memset(cT[:], 0.0)
                nc.gpsimd.memset(hT[:], 0.0)

                recur_layer(tc, pools, T, WDT,
                            xsrc=[(locT, k) for k in range(4)]
                                 + [(pbuf, k) for k in range(4)],
                            wihT=[(w1oT, k) for k in range(4)]
                                 + [(w1pT, k) for k in range(4)],
                            whhT=whh1T, biasT=b1T)

                nc.sync.dma_start(h1[:], hT[:])

    nc.compile()
    return nc


def _xslice(src, k, col, n=128):
    """[128, n] moving slice at column `col` from an x-like source."""
    t, kk = src
    if len(t.shape) == 3 and t.shape[0] == 4:     # x_T [4, 128, TBP]
        return t[kk, :, col] if isinstance(col, slice) else t[kk, :, ds(col, n)]
    # locT/pbuf [128, 4, TBP]
    return t[:, kk, col] if isinstance(col, slice) else t[:, kk, ds(col, n)]


def recur_layer(tc, P, T, WDT, xsrc, wihT, whhT, biasT, locT=None, revT=None,
                dbg_pre=None):
    """One LSTM layer: fused projection + recurrence.

    xsrc: list of (dram_tensor, k) moving-operand sources, one per 128-wide
    contraction chunk (4 for layer 0, 4 own + 4 partner for layer 1).
    """
    nc = tc.nc
    nk = len(xsrc)
    store = locT is not None
    bank_ig, bank_fo = P["bank_ig"], P["bank_fo"]
    cT, hT, pres = P["cT"], P["hT"], P["pres"]
    gp, stp, xp, ppsum, op = P["gp"], P["stp"], P["xp"], P["ppsum"], P["op"]
    REV = 8 * T - 8

    with tc.tile_pool(name="wpool", bufs=1) as wp:
        whh_sb = []
        for k in range(4):
            w = wp.tile([128, G4], WDT, name=f"whh{k}")
            nc.sync.dma_start(w[:], whhT[k])
            whh_sb.append(w)
        wih_sb = []
        for j, (t, kk) in enumerate(wihT):
            w = wp.tile([128, G4], BF16, name=f"wih{j}")
            nc.sync.dma_start(w[:], t[kk])
            wih_sb.append(w)
        bias_sb = wp.tile([128, 16], F32)
        nc.sync.dma_start(bias_sb[:], biasT[:])

        def dma_xchunk(parity, col):
            """Fetch the [128,128] moving tiles for one chunk's projection."""
            xts = []
            for j in range(nk):
                xt = xp.tile([128, 128], BF16, name=f"x{j}", tag=f"x{j}p{parity}")
                nc.sync.dma_start(xt[:], _xslice(xsrc[j], j, col))
                xts.append(xt)
            return xts

        def emit_proj(s, xts, dst, pps):
            """Projection matmuls for mslot s into pps col 128*(s%4)."""
            m = MORDER[s]
            col = 128 * (s % 4)
            for j in range(nk):
                nc.tensor.matmul(pps[:, col:col + 128],
                                 wih_sb[j][:, 128 * m:128 * (m + 1)], xts[j][:],
                                 start=(j == 0), stop=(j == nk - 1))

        def emit_proj_copies(g, dst, pps):
            """Evacuate mslots 4g..4g+3 (+bias) from pps into dst."""
            for jj in range(4):
                s = 4 * g + jj
                nc.vector.tensor_scalar_add(dst[:, s, :],
                                            pps[:, 128 * jj:128 * jj + 128],
                                            bias_sb[:, s:s + 1])

        def prewrite(nxt_s):
            """DVE-overwrite both gate banks with pre for step nxt_s."""
            buf = pres[(nxt_s // CH) % 4]
            c0 = 8 * (nxt_s % CH)
            nc.vector.tensor_copy(
                bank_ig[:].rearrange("p (s b) -> p s b", s=8),
                buf[:, 0:8, c0:c0 + 8])
            nc.vector.tensor_copy(
                bank_fo[:].rearrange("p (s b) -> p s b", s=8),
                buf[:, 8:16, c0:c0 + 8])

        def emit_step(u, iv8, xts, dst, pps):
            """One LSTM step; consumes prewritten banks, prewrites step u+1.

            u: step index within the body (0..UNROLL-1); also emits the
            projection for mslot (u%CH) of the lookahead chunk into dst.
            """
            # gate matmuls: bank_ig = [i0..3 | g0..3]
            for j in range(4):
                for k in range(4):
                    nc.tensor.matmul(bank_ig[:, 8 * j:8 * j + 8],
                                     whh_sb[k][:, 128 * j:128 * (j + 1)],
                                     hT[:, 8 * k:8 * k + 8],
                                     start=False, stop=False, skip_group_check=True)
            for j in range(4):
                m = 8 + j
                for k in range(4):
                    nc.tensor.matmul(bank_ig[:, 32 + 8 * j:40 + 8 * j],
                                     whh_sb[k][:, 128 * m:128 * (m + 1)],
                                     hT[:, 8 * k:8 * k + 8],
                                     start=False, stop=False, skip_group_check=True)
            # emit the ig activations BEFORE the fo matmuls: the PE->ACT
            # semaphore inc then lands right after the last ig matmul, so
            # tanh(g)/sig(i)/a run UNDER the fo matmul stream.
            tg = gp.tile([128, 32], F32, tag="tg")
            nc.scalar.activation(tg[:], bank_ig[:, 32:64], AF.Tanh, scale=0.0625)
            si = gp.tile([128, 32], F32, tag="si")
            nc.scalar.activation(si[:], bank_ig[:, 0:32], AF.Sigmoid, scale=0.0625)

            # bank_fo = [f0..3 | o0..3]
            for j in range(4):
                m = 4 + j
                for k in range(4):
                    nc.tensor.matmul(bank_fo[:, 8 * j:8 * j + 8],
                                     whh_sb[k][:, 128 * m:128 * (m + 1)],
                                     hT[:, 8 * k:8 * k + 8],
                                     start=False, stop=False, skip_group_check=True)
            for j in range(4):
                m = 12 + j
                for k in range(4):
                    nc.tensor.matmul(bank_fo[:, 32 + 8 * j:40 + 8 * j],
                                     whh_sb[k][:, 128 * m:128 * (m + 1)],
                                     hT[:, 8 * k:8 * k + 8],
                                     start=False, stop=False, skip_group_check=True)
            sfo = gp.tile([128, 64], F32, tag="sfo")
            nc.scalar.activation(sfo[:], bank_fo[:], AF.Sigmoid, scale=0.0625)

            a = gp.tile([128, 32], F32, tag="a")
            nc.vector.tensor_tensor(a[:], si[:], tg[:], MULT)
            # prewrite_ig as early as possible (only needs si/tg reads done)
            buf = pres[((u + 1) // CH) % 4]
            c0 = 8 * ((u + 1) % CH)
            nc.vector.tensor_copy(
                bank_ig[:].rearrange("p (s b) -> p s b", s=8),
                buf[:, 0:8, c0:c0 + 8])
            btmp = gp.tile([128, 32], F32, tag="btmp")
            nc.vector.tensor_tensor(btmp[:], sfo[:, 0:32], cT[:], MULT)
            nc.vector.tensor_tensor(cT[:], a[:], btmp[:], ADD)
            # projection filler: emitted after the gate ACTs (so their PE
            # semaphore incs land right after each bank's last matmul) but
            # before the proj copies (program-order dependency tracking).
            emit_proj(u % CH, xts, dst, pps)
            # evacuate the projection PSUM group every 4 mslots.  Must be
            # emitted before prewrite_fo: at u%CH==15 the prewrite reads pre
            # slots this group produces, and the DVE queue is strict FIFO.
            if u % 4 == 3:
                emit_proj_copies((u % CH) // 4, dst, pps)
            nc.vector.tensor_copy(
                bank_fo[:].rearrange("p (s b) -> p s b", s=8),
                buf[:, 8:16, c0:c0 + 8])
            tct = gp.tile([128, 32], F32, tag="tct")
            nc.scalar.activation(tct[:], cT[:], AF.Tanh)
            nc.vector.tensor_tensor(hT[:], sfo[:, 32:64], tct[:], MULT)

            if store:
                st = stp.tile([128, 32], BF16, tag="st")
                nc.gpsimd.tensor_copy(st[:], hT[:])
                st3 = st[:].rearrange("p (k b) -> p k b", k=4)
                nc.sync.dma_start(locT[:, :, ds(iv8, 8)], st3)
                nc.gpsimd.dma_start(revT[:, :, ds(REV - iv8, 8)], st3)

        # ---- prologue: produce chunks 0-1 into pres[0:2], prewrite step 0 ----
        for c in range(2):
            xtsp = dma_xchunk(c, slice(128 * c, 128 * c + 128))
            for g in range(4):
                pps = ppsum.tile([128, 512], F32, tag="pj", name="pps")
                for jj in range(4):
                    emit_proj(4 * g + jj, xtsp, pres[c], pps)
                emit_proj_copies(g, pres[c], pps)
        prewrite(0)
        if dbg_pre is not None:
            nc.sync.dma_start(dbg_pre[:], pres[0][:])

        # ---- main loop: 32 steps (2 chunks) per body ----
        with tc.For_i(0, 8 * T, 8 * UNROLL,
                      hint_engines=(mybir.EngineType.PE,)) as iv0:
            # lookahead-2 chunk DMAs: quarter q produces chunk c+q+2 into
            # pres[(q+2)%4] (c = body's first chunk)
            xts_q = [dma_xchunk((q + 2) % 4, iv0 + 8 * CH * (q + 2))
                     for q in range(4)]
            pps = None
            for u in range(UNROLL):
                q = u // CH
                if u % 4 == 0:
                    pps = ppsum.tile([128, 512], F32, tag="pj", name="pps")
                emit_step(u, iv0 + 8 * u, xts_q[q], pres[(q + 2) % 4], pps)


# ----------------------------------------------------------------------------
# Host side
# ----------------------------------------------------------------------------

_PROG_CACHE = {}


def _get_program(T):
    if T not in _PROG_CACHE:
        _PROG_CACHE[T] = build_program(T)
    return _PROG_CACHE[T]


def _bf16(a):
    import ml_dtypes
    return np.asarray(a, np.float32).astype(ml_dtypes.bfloat16)


def _fp8(a):
    import ml_dtypes
    return np.asarray(a, np.float32).astype(ml_dtypes.float8_e4m3)


SCALE = 16.0


def _chunkT(w, dtype_fn=_bf16, scale=SCALE):
    """[G4, K] weight -> scaled W^T [K//128, 128, G4]."""
    wt = np.ascontiguousarray(w.T.astype(np.float32)) * scale
    return dtype_fn(wt.reshape(wt.shape[0] // 128, 128, w.shape[0]))


def _biasT(b_ih, b_hh, scale=SCALE):
    """-> [128, 16] f32 per-mslot per-partition bias (x scale)."""
    b = (np.asarray(b_ih, np.float32) + np.asarray(b_hh, np.float32)) * scale
    out = np.zeros((128, 16), np.float32)
    for s, m in enumerate(MORDER):
        out[:, s] = b[128 * m:128 * (m + 1)]
    return out


def _prep_inputs(x, w_ih_f0, w_hh_f0, b_ih_f0, b_hh_f0,
                 w_ih_b0, w_hh_b0, b_ih_b0, b_hh_b0,
                 w_ih_f1, w_hh_f1, b_ih_f1, b_hh_f1,
                 w_ih_b1, w_hh_b1, b_ih_b1, b_hh_b1,
                 mask, T):
    f32 = np.float32
    wq = _fp8 if USE_FP8 else _bf16
    whh_f0m = (w_hh_f0 * mask).astype(f32)

    per_dir = {
        0: dict(w0T=_chunkT(w_ih_f0), b0T=_biasT(b_ih_f0, b_hh_f0),
                whh0T=_chunkT(whh_f0m, wq),
                w1oT=_chunkT(w_ih_f1[:, :H]), w1pT=_chunkT(w_ih_f1[:, H:]),
                b1T=_biasT(b_ih_f1, b_hh_f1),
                whh1T=_chunkT(w_hh_f1, wq)),
        1: dict(w0T=_chunkT(w_ih_b0), b0T=_biasT(b_ih_b0, b_hh_b0),
                whh0T=_chunkT(w_hh_b0, wq),
                w1oT=_chunkT(w_ih_b1[:, H:]), w1pT=_chunkT(w_ih_b1[:, :H]),
                b1T=_biasT(b_ih_b1, b_hh_b1),
                whh1T=_chunkT(w_hh_b1, wq)),
    }

    TB = T * BS
    in_maps = []
    for core in range(NCORES):
        pair, q = core // 2, core % 2
        xs = x[pair * BS:(pair + 1) * BS, :T].astype(f32)   # [8, T, E]
        if q == 1:
            xs = xs[:, ::-1]
        xT = np.ascontiguousarray(xs.transpose(2, 1, 0)).reshape(4, 128, TB)
        xTp = np.zeros((4, 128, TB + PAD), np.float32)
        xTp[:, :, :TB] = xT
        m = dict(per_dir[q])
        m["x_T"] = _bf16(xTp)
        m["pslot"] = np.array([[1 - q]], dtype=np.uint32)
        in_maps.append(m)
    return in_maps


def _mish(x):
    return x * np.tanh(np.log1p(np.exp(-np.abs(x))) + np.maximum(x, 0.0))


def _unT(hT):
    """[128, 32] h^T-chunk layout -> [8, 512]: h[b, 128k+p] = hT[p, 8k+b]."""
    a = np.asarray(hT, np.float32).reshape(128, 4, 8)     # p, k, b
    return np.ascontiguousarray(a.transpose(2, 1, 0)).reshape(8, 512)


def _head(h1s, fc_w, fc_b):
    h1s = [_unT(h) for h in h1s]
    h_f = np.concatenate([np.asarray(h1s[2 * p], np.float32) for p in range(4)], axis=0)
    h_b = np.concatenate([np.asarray(h1s[2 * p + 1], np.float32) for p in range(4)], axis=0)
    h = 0.5 * (h_f + h_b)
    z = h @ np.asarray(fc_w, np.float32).T + np.asarray(fc_b, np.float32)
    return _mish(z).astype(np.float32)


def run_device(inputs, trace=False, tmpdir=None):
    from concourse.bass_utils import run_bass_kernel_spmd

    x = inputs["x"]
    T = x.shape[1]
    nc = _get_program(T)
    in_maps = _prep_inputs(
        x, inputs["w_ih_f0"], inputs["w_hh_f0"], inputs["b_ih_f0"], inputs["b_hh_f0"],
        inputs["w_ih_b0"], inputs["w_hh_b0"], inputs["b_ih_b0"], inputs["b_hh_b0"],
        inputs["w_ih_f1"], inputs["w_hh_f1"], inputs["b_ih_f1"], inputs["b_hh_f1"],
        inputs["w_ih_b1"], inputs["w_hh_b1"], inputs["b_ih_b1"], inputs["b_hh_b1"],
        inputs["mask"], T)

    res = run_bass_kernel_spmd(nc, in_maps, list(range(NCORES)),
                               trace=trace, tmpdir=tmpdir)
    h1s = [res.results[c]["h1"] for c in range(NCORES)]
    return h1s, res


def kernel(**inputs):
    h1s, _ = run_device(inputs)
    return _head(h1s, inputs["fc_w"], inputs["fc_b"])
